# revision 20
# baseline (speedup 1.0000x reference)
"""2-layer GAT (heads=4, concat=False, ELU between) on 8 Trainium2 cores.

Design v2 (batched edge sweep, no one-hot transpose machinery):

Row space: 51200 rows = nodes 0..32639 | 1024 zero rows | nodes 32640..50175.
  row(n) = n + 1024*(n >= 32640). The zero gap gives every int16 gather
  window a known-zero row for junk indices.
xcat row (768B, 384 f16): [xh 256 | als 4 | ald 4 | pad 120].

Per layer:
- Dense phase (replicated): 50 batches x 1024 rows, xT via transpose-DMA,
  8 matmuls vs packed [fin,264] weights, PSUM -> xcat f16 rows.
- ald_loc build: two gathers (lo/hi windows of xcat cols 256:384, per-core
  indices select this core's 6272 slots; junk side hits zero rows), DVE add,
  store ald_loc [6272, 128] f16 (slot-major).
- Edge sweep per dst tile (128 nodes, C[t] chunks of 128 edges):
  3 gathers: G src rows (768B, lo/hi) + per-edge ald (256B rows of
  ald_loc tile window, idx = local dst).
  Batched attention: alpha = als+ald (TT), lrelu (STT), broadcast-exp on
  ACT -> wexp [128,C,4,64] f16, G *= wexp in place (TT), w -> cols 260:264.
  Per chunk: one f16 is_equal one-hot + one f16 matmul
  agg[128,264] += sel^T @ G[:,c,0:264] (denominators ride cols 260:264).
  Epilogue: recip*0.25, 4 ACT head scales, reduce over heads,
  ELU = max(s, exp(min(s,0))-1). h AllGathered as f16 [*,64].
"""
import sys
import os

sys.path.insert(0, '/opt/pypackages')
sys.path.insert(0, '/opt/trn_rl_repo')

import numpy as np

import concourse.bacc as bacc
import concourse.mybir as mybir
import concourse.tile as tile
from concourse.bass_utils import run_bass_kernel_spmd

F16 = mybir.dt.float16
F32 = mybir.dt.float32
I16 = mybir.dt.int16

NEG_SLOPE = 0.2


class Cfg:
    def __init__(self):
        self.N = 50000
        self.IN = 128
        self.H = 64
        self.OUT = 64
        self.HEADS = 4
        self.NCORES = 8
        self.T = 49                      # dst tiles per core
        self.NPC = self.T * 128          # 6272 slots per core
        self.NP = self.NCORES * self.NPC  # 50176 slots
        self.GAP_AT = 32640              # zero-gap insertion point (rows)
        self.GAP = 1024
        self.R = self.NP + self.GAP      # 51200 xcat rows
        self.SPLIT = 32768               # row-space int16 split
        self.ROW = 384                   # f16 elems per row (768B)


FULL = Cfg()


def _row(n):
    """node/slot id -> xcat row id (insert zero gap)."""
    n = np.asarray(n, dtype=np.int64)
    return n + np.where(n >= FULL.GAP_AT, FULL.GAP, 0)


def _wrap16(idx):
    """[n] int array (n%16==0) -> [128, n//16] int16 gather layout."""
    n = len(idx)
    base = np.asarray(idx, dtype=np.int16).reshape(n // 16, 16).T
    return np.tile(base, (8, 1))


def host_prep(cfg, edge_index):
    src = np.asarray(edge_index[0], dtype=np.int64)
    dst = np.asarray(edge_index[1], dtype=np.int64)
    loops = np.arange(cfg.N, dtype=np.int64)
    src = np.concatenate([src, loops])
    dst = np.concatenate([dst, loops])

    core_of = dst // cfg.NPC
    tile_of = (dst % cfg.NPC) // 128

    order = np.lexsort((src, tile_of, core_of))
    src_s, dst_s = src[order], dst[order]
    key = core_of[order] * cfg.T + tile_of[order]
    starts = np.searchsorted(key, np.arange(cfg.NCORES * cfg.T), side='left')
    ends = np.searchsorted(key, np.arange(cfg.NCORES * cfg.T), side='right')

    C_lo = np.zeros(cfg.T, dtype=np.int64)
    C_hi = np.zeros(cfg.T, dtype=np.int64)
    lists = {}
    for c in range(cfg.NCORES):
        for t in range(cfg.T):
            k = c * cfg.T + t
            es, ed = src_s[starts[k]:ends[k]], dst_s[starts[k]:ends[k]]
            lo = es < cfg.GAP_AT
            lists[(c, t)] = (es[lo], ed[lo], es[~lo], ed[~lo])
            C_lo[t] = max(C_lo[t], (int(np.sum(lo)) + 127) // 128)
            C_hi[t] = max(C_hi[t], (len(es) - int(np.sum(lo)) + 127) // 128)
    C_lo = np.maximum(C_lo, 1)
    C_hi = np.maximum(C_hi, 1)
    C = C_lo + C_hi
    TOT = int(C.sum())

    gidx = np.zeros((cfg.NCORES, 128, TOT * 8), dtype=np.int16)
    dst2 = np.zeros((cfg.NCORES, 2, TOT * 128), dtype=np.float16)
    dst2[:, 0, :] = 1.0
    dstloc = np.full((cfg.NCORES, 128, TOT), -1.0, dtype=np.float32)
    off = np.zeros(cfg.T + 1, dtype=np.int64)
    off[1:] = np.cumsum(C)
    for c in range(cfg.NCORES):
        for t in range(cfg.T):
            base = (c * cfg.T + t) * 128
            es_lo, ed_lo, es_hi, ed_hi = lists[(c, t)]
            nlo, nhi = int(C_lo[t]) * 128, int(C_hi[t]) * 128
            gi = np.zeros(nlo + nhi, dtype=np.int64)
            gi[:len(es_lo)] = es_lo                      # row(src)=src (<32640)
            gi[nlo:nlo + len(es_hi)] = es_hi + cfg.GAP - cfg.SPLIT
            dl = np.full(nlo + nhi, -1.0, dtype=np.float32)
            dl[:len(ed_lo)] = ed_lo - base
            dl[nlo:nlo + len(ed_hi)] = ed_hi - base
            o = int(off[t])
            ct = int(C[t])
            gidx[c, :, o * 8:(o + ct) * 8] = _wrap16(gi)
            dst2[c, 1, o * 128:(o + ct) * 128] = (-dl).astype(np.float16)
            dstloc[c, :, o:o + ct] = \
                dl.reshape(ct, 128).T.astype(np.float32)

    # per-core ald_loc build indices: slot i -> node c*NPC+i
    aldc_lo = np.zeros((cfg.NCORES, 128, cfg.NPC // 16), dtype=np.int16)
    aldc_hi = np.zeros((cfg.NCORES, 128, cfg.NPC // 16), dtype=np.int16)
    for c in range(cfg.NCORES):
        nodes = c * cfg.NPC + np.arange(cfg.NPC)
        is_lo = nodes < cfg.GAP_AT
        ilo = np.where(is_lo, nodes, cfg.GAP_AT)          # junk -> zero row
        ihi = np.where(is_lo, 0, nodes + cfg.GAP - cfg.SPLIT)
        aldc_lo[c] = _wrap16(ilo)
        aldc_hi[c] = _wrap16(ihi)
    return dict(C_lo=C_lo, C_hi=C_hi, C=C, off=off, TOT=TOT,
                gidx=gidx, dst2=dst2, dstloc=dstloc,
                aldc_lo=aldc_lo, aldc_hi=aldc_hi)


def _weights_cat(W, a_src, a_dst, heads, ch):
    """[Fin, heads*ch] + [heads, ch]x2 -> fp16 [Fin, 264]."""
    fin = W.shape[0]
    ws = np.einsum('fhc,hc->fh', W.reshape(fin, heads, ch), a_src)
    wd = np.einsum('fhc,hc->fh', W.reshape(fin, heads, ch), a_dst)
    out = np.zeros((fin, 264), dtype=np.float16)
    out[:, :heads * ch] = W.astype(np.float16)
    out[:, 256:260] = ws.astype(np.float16)
    out[:, 260:264] = wd.astype(np.float16)
    return out


def build_kernel(cfg, C_lo, C_hi, C, off, TOT):
    nc = bacc.Bacc("TRN2", target_bir_lowering=False, debug=False,
                   num_devices=cfg.NCORES, num_swdge_queues=4)
    R = cfg.R
    NB = R // 1024  # 50 dense batches

    x_in = nc.dram_tensor("x16t", [cfg.IN, R], F16, kind="ExternalInput")
    wa1 = nc.dram_tensor("wa1", [cfg.IN, 264], F16, kind="ExternalInput")
    wa2 = nc.dram_tensor("wa2", [cfg.H, 264], F16, kind="ExternalInput")
    mconst = nc.dram_tensor("mconst", [128, 128], F32, kind="ExternalInput")
    ident = nc.dram_tensor("ident", [128, 128], F16, kind="ExternalInput")
    gidx_d = nc.dram_tensor("gidx", [128, TOT * 8], I16, kind="ExternalInput")
    dst2_d = nc.dram_tensor("dst2", [2, TOT * 128], F16,
                            kind="ExternalInput")
    pio2 = nc.dram_tensor("pio2", [2, 128], F16, kind="ExternalInput")
    dstloc_d = nc.dram_tensor("dstloc", [128, TOT], F32,
                              kind="ExternalInput")
    aldc_lo_d = nc.dram_tensor("aldc_lo", [128, cfg.NPC // 16], I16,
                               kind="ExternalInput")
    aldc_hi_d = nc.dram_tensor("aldc_hi", [128, cfg.NPC // 16], I16,
                               kind="ExternalInput")
    out_d = nc.dram_tensor("out_slice", [cfg.NPC, cfg.OUT], F32,
                           kind="ExternalOutput")

    with tile.TileContext(nc) as tc:
        with tc.tile_pool(name="dram", bufs=1, space="DRAM") as dpool, \
             tc.tile_pool(name="const", bufs=1) as cpool, \
             tc.tile_pool(name="dense", bufs=3) as dnpool, \
             tc.tile_pool(name="aldb", bufs=1) as abpool, \
             tc.tile_pool(name="work", bufs=3) as pool, \
             tc.tile_pool(name="gpool", bufs=4) as gpool, \
             tc.tile_pool(name="apool", bufs=3) as apool, \
             tc.tile_pool(name="wpool", bufs=3) as wpool, \
             tc.tile_pool(name="seld", bufs=6) as sdpool, \
             tc.tile_pool(name="psA", bufs=2, space="PSUM") as psA, \
             tc.tile_pool(name="psB", bufs=2, space="PSUM") as psB, \
             tc.tile_pool(name="psU", bufs=1, space="PSUM") as psU, \
             tc.tile_pool(name="psL", bufs=1, space="PSUM") as psL:

            xcat1 = dpool.tile([R, cfg.ROW], F16, name="xcat1",
                               uniquify=False)
            xcat2 = dpool.tile([R, cfg.ROW], F16, name="xcat2",
                               uniquify=False)
            ald1 = dpool.tile([cfg.NPC, 128], F16, name="ald1",
                              uniquify=False)
            ald2 = dpool.tile([cfg.NPC, 128], F16, name="ald2",
                              uniquify=False)
            h_loc = dpool.tile([64, cfg.NPC], F16, name="h_loc",
                               uniquify=False)
            h_full = dpool.tile([cfg.NCORES, 64, cfg.NPC], F16,
                                name="h_full", uniquify=False,
                                addr_space="Shared")

            mconst_sb = cpool.tile([128, 128], F32)
            nc.sync.dma_start(out=mconst_sb[:], in_=mconst[:, :])
            ident_sb = cpool.tile([128, 128], F16)
            nc.sync.dma_start(out=ident_sb[:], in_=ident[:, :])
            wa1_sb = cpool.tile([cfg.IN, 264], F16)
            nc.sync.dma_start(out=wa1_sb[:], in_=wa1[:, :])
            wa2_sb = cpool.tile([cfg.H, 264], F16)
            nc.sync.dma_start(out=wa2_sb[:], in_=wa2[:, :])
            aldc_lo_sb = cpool.tile([128, cfg.NPC // 16], I16)
            nc.sync.dma_start(out=aldc_lo_sb[:], in_=aldc_lo_d[:, :])
            aldc_hi_sb = cpool.tile([128, cfg.NPC // 16], I16)
            nc.sync.dma_start(out=aldc_hi_sb[:], in_=aldc_hi_d[:, :])
            pio2_sb = cpool.tile([2, 128], F16)
            nc.sync.dma_start(out=pio2_sb[:], in_=pio2[:, :])
            zero_sb = cpool.tile([128, 1024], F16)
            nc.gpsimd.memset(zero_sb[:], 0)

            def dense_phase(src_h, fin, wa_sb, xcat, layer):
                """layer1: x_in is [IN, R] pre-transposed; layer2: h_full
                is [8, 64, NPC] (slot-major columns)."""
                GA, GE = cfg.GAP_AT, cfg.GAP_AT + cfg.GAP
                for b in range(NB):
                    nb = b * 1024
                    xT = dnpool.tile([128, 1024], F16, name=f"xT{layer}_{b}",
                                     tag="xT")
                    if layer == 1:
                        nc.sync.dma_start(out=xT[0:fin, :],
                                          in_=x_in[:, nb:nb + 1024])
                    else:
                        # xcat row j -> slot j (j<GA), zero (GA<=j<GE),
                        # else j-GAP; slots map to h_full[c, :, local]
                        segs = []
                        j = nb
                        while j < nb + 1024:
                            if j < GA:
                                n = min(GA, nb + 1024) - j
                                segs.append((j - nb, n, j))
                            elif j < GE:
                                n = min(GE, nb + 1024) - j
                                segs.append((j - nb, n, None))
                            else:
                                n = nb + 1024 - j
                                segs.append((j - nb, n, j - cfg.GAP))
                            j += n
                        for (xo, n, s0) in segs:
                            if s0 is None:
                                nc.vector.tensor_copy(
                                    xT[0:fin, xo:xo + n],
                                    zero_sb[0:fin, 0:n])
                                continue
                            while n > 0:
                                c0 = s0 // cfg.NPC
                                l0 = s0 - c0 * cfg.NPC
                                nn = min(n, cfg.NPC - l0)
                                nc.sync.dma_start(
                                    out=xT[0:fin, xo:xo + nn],
                                    in_=src_h[c0, :, l0:l0 + nn])
                                xo += nn
                                s0 += nn
                                n -= nn
                    xc = dnpool.tile([128, 8, 264], F16,
                                     name=f"xc{layer}_{b}", tag="xc")
                    for s in range(8):
                        ps = psA.tile([128, 264], F32,
                                      name=f"dp{layer}_{b}_{s}", tag="dps")
                        nc.tensor.matmul(
                            ps[:], xT[0:fin, s * 128:(s + 1) * 128],
                            wa_sb[:, :], start=True, stop=True)
                        nc.scalar.activation(
                            xc[:, s, :], ps[:, :],
                            mybir.ActivationFunctionType.Copy)
                    for half in range(2):
                        nc.sync.dma_start(
                            out=xcat[nb + half * 512:nb + half * 512 + 512,
                                     0:264].rearrange(
                                "(s p) d -> p s d", p=128),
                            in_=xc[:, half * 4:(half + 1) * 4, :])

            def ald_build(xcat, ald_loc, layer):
                """Collect this core's slots' [als|ald|pad] windows."""
                a_lo = abpool.tile([128, cfg.T, 128], F16,
                                   name=f"alo{layer}", tag="alo")
                nc.gpsimd.dma_gather(
                    a_lo[:], xcat[0:cfg.SPLIT, 256:384], aldc_lo_sb[:],
                    cfg.NPC, cfg.NPC, 128, elem_step=cfg.ROW,
                    single_packet=False, queue_num=0)
                a_hi = abpool.tile([128, cfg.T, 128], F16,
                                   name=f"ahi{layer}", tag="ahi")
                nc.gpsimd.dma_gather(
                    a_hi[:], xcat[cfg.SPLIT:R, 256:384], aldc_hi_sb[:],
                    cfg.NPC, cfg.NPC, 128, elem_step=cfg.ROW,
                    single_packet=False, queue_num=1)
                a_sum = abpool.tile([128, cfg.T, 128], F16,
                                    name=f"asm{layer}", tag="asm")
                nc.vector.tensor_tensor(out=a_sum[:], in0=a_lo[:],
                                        in1=a_hi[:],
                                        op=mybir.AluOpType.add)
                return a_sum

            def edge_sweep(xcat, a_sum, layer):
                for t in range(cfg.T):
                    q = (2 * t) % 4
                    q2 = (2 * t + 1) % 4
                    sfx = f"_{layer}_{t}"
                    Ct = int(C[t])
                    Clo = int(C_lo[t])
                    o = int(off[t])
                    idx_t = pool.tile([128, Ct * 8], I16, name="ix" + sfx,
                                      tag="ix")
                    nc.sync.dma_start(out=idx_t[:],
                                      in_=gidx_d[:, o * 8:(o + Ct) * 8])
                    d2_t = pool.tile([2, Ct * 128], F16, name="d2" + sfx,
                                     tag="d2")
                    nc.sync.dma_start(
                        out=d2_t[:],
                        in_=dst2_d[:, o * 128:(o + Ct) * 128])
                    dst_t = pool.tile([128, Ct], F32, name="dl" + sfx,
                                      tag="dl")
                    nc.sync.dma_start(out=dst_t[:],
                                      in_=dstloc_d[:, o:o + Ct])

                    G = gpool.tile([128, Ct, cfg.ROW], F16, name="G" + sfx,
                                   tag="G")
                    nc.gpsimd.dma_gather(
                        G[:, 0:Clo, :], xcat[0:cfg.SPLIT, :],
                        idx_t[:, 0:Clo * 8], Clo * 128, Clo * 128,
                        cfg.ROW, single_packet=False, queue_num=q)
                    nc.gpsimd.dma_gather(
                        G[:, Clo:Ct, :], xcat[cfg.SPLIT:R, :],
                        idx_t[:, Clo * 8:], (Ct - Clo) * 128,
                        (Ct - Clo) * 128,
                        cfg.ROW, single_packet=False, queue_num=q2)
                    # U[d,e] = d - dst[e] via K=2 matmul; SDT = (U == 0)
                    PC = ((Ct + 2) // 3) * 128
                    pieces = []
                    pb = 0
                    while pb < Ct * 128:
                        pieces.append((pb, min(PC, Ct * 128 - pb)))
                        pb += PC
                    sdt = apool.tile([128, Ct * 128], F16, name="st" + sfx,
                                     tag="st")
                    for pi, (pb, pn) in enumerate(pieces):
                        if pn <= 0:
                            continue
                        ups = psU.tile([128, 896], F32,
                                       name=f"up{sfx}_{pi}", tag="up")
                        for n0 in range(0, pn, 512):
                            nn = min(512, pn - n0)
                            nc.tensor.matmul(
                                ups[:, n0:n0 + nn], pio2_sb[:],
                                d2_t[:, pb + n0:pb + n0 + nn],
                                start=True, stop=True,
                                skip_group_check=True)
                        nc.vector.tensor_scalar(
                            sdt[:, pb:pb + pn], ups[:, 0:pn], 0.0, None,
                            mybir.AluOpType.is_equal)
                    ald_ps = psL.tile([128, Ct * 4], F32, name="ap" + sfx,
                                      tag="ap")
                    for c in range(Ct):
                        nc.tensor.matmul(
                            ald_ps[:, c * 4:(c + 1) * 4],
                            sdt[:, c * 128:(c + 1) * 128],
                            a_sum[:, t, 4:8], start=True, stop=True,
                            skip_group_check=True)
                    alde = pool.tile([128, Ct, 4], F16, name="ae" + sfx,
                                     tag="ae")
                    nc.vector.tensor_copy(
                        alde[:], ald_ps[:].rearrange("p (c f) -> p c f",
                                                     f=4))

                    alpha = pool.tile([128, Ct, 4], F32, name="al" + sfx,
                                      tag="al")
                    nc.vector.tensor_tensor(
                        out=alpha[:], in0=G[:, :, 256:260],
                        in1=alde[:], op=mybir.AluOpType.add)
                    wpre = pool.tile([128, Ct, 4], F32, name="wp" + sfx,
                                     tag="wp")
                    nc.vector.scalar_tensor_tensor(
                        out=wpre[:], in0=alpha[:], scalar=NEG_SLOPE,
                        in1=alpha[:], op0=mybir.AluOpType.mult,
                        op1=mybir.AluOpType.max)
                    wexp = wpool.tile([128, Ct, 4, 64], F16, name="we" + sfx,
                                      tag="we")
                    nc.scalar.activation(
                        wexp[:], wpre[:].unsqueeze(3).to_broadcast(
                            [128, Ct, 4, 64]),
                        mybir.ActivationFunctionType.Exp)
                    nc.vector.tensor_copy(
                        G[:, :, 260:264], wexp[:, :, :, 0])
                    nc.vector.tensor_tensor(
                        out=G[:, :, 0:256].rearrange(
                            "p c (h f) -> p c h f", h=4),
                        in0=G[:, :, 0:256].rearrange(
                            "p c (h f) -> p c h f", h=4),
                        in1=wexp[:], op=mybir.AluOpType.mult)

                    agg = psB.tile([128, 264], F32, name="agg" + sfx,
                                   tag="agg")
                    for c in range(Ct):
                        sel = sdpool.tile([128, 128], F16,
                                          name=f"sd{sfx}_{c}", tag="sd")
                        nc.vector.tensor_scalar(
                            sel[:], mconst_sb[:], dst_t[:, c:c + 1], None,
                            mybir.AluOpType.is_equal)
                        nc.tensor.matmul(
                            agg[:], sel[:], G[:, c, 0:264],
                            start=(c == 0), stop=(c == Ct - 1),
                            skip_group_check=True)

                    den = pool.tile([128, 4], F32, name="dn" + sfx, tag="dn")
                    nc.vector.tensor_scalar(den[:], agg[:, 260:264], 1e-16,
                                            None, mybir.AluOpType.max)
                    rec = pool.tile([128, 4], F32, name="rc" + sfx, tag="rc")
                    nc.vector.reciprocal(rec[:], den[:])
                    nc.vector.tensor_scalar(rec[:], rec[:], 0.25, None,
                                            mybir.AluOpType.mult)
                    tmp = pool.tile([128, 64, 4], F32, name="tm" + sfx,
                                    tag="tm")
                    for h in range(4):
                        nc.scalar.activation(
                            tmp[:, :, h], agg[:, h * 64:(h + 1) * 64],
                            mybir.ActivationFunctionType.Copy,
                            scale=rec[:, h:h + 1])
                    s0 = pool.tile([128, 64], F32, name="s0" + sfx, tag="s0")
                    nc.vector.tensor_reduce(
                        s0[:], tmp[:],
                        mybir.AxisListType.X, mybir.AluOpType.add)
                    if layer == 1:
                        ng = pool.tile([128, 64], F32, name="ng" + sfx,
                                       tag="ng")
                        nc.vector.tensor_scalar(ng[:], s0[:], 0.0, None,
                                                mybir.AluOpType.min)
                        ex = pool.tile([128, 64], F32, name="ex" + sfx,
                                       tag="ex")
                        nc.scalar.activation(
                            ex[:], ng[:], mybir.ActivationFunctionType.Exp)
                        hc = pool.tile([128, 64], F16, name="hc" + sfx,
                                       tag="hc")
                        nc.vector.scalar_tensor_tensor(
                            out=hc[:], in0=ex[:], scalar=1.0,
                            in1=s0[:],
                            op0=mybir.AluOpType.subtract,
                            op1=mybir.AluOpType.max)
                        trp = psL.tile([64, 128], F16, name="tr" + sfx,
                                       tag="tr")
                        nc.tensor.transpose(trp[:], hc[:], ident_sb[:])
                        hT = pool.tile([64, 128], F16, name="hT" + sfx,
                                       tag="hT")
                        nc.vector.tensor_copy(hT[:], trp[:])
                        nc.sync.dma_start(
                            out=h_loc[:, t * 128:(t + 1) * 128], in_=hT[:])
                    else:
                        nc.sync.dma_start(
                            out=out_d[t * 128:(t + 1) * 128, :], in_=s0[:])

            dense_phase(None, cfg.IN, wa1_sb, xcat1, 1)
            asum1 = ald_build(xcat1, ald1, 1)
            edge_sweep(xcat1, asum1, 1)
            nc.gpsimd.collective_compute(
                "AllGather", mybir.AluOpType.bypass,
                replica_groups=[list(range(cfg.NCORES))],
                ins=[h_loc.opt()], outs=[h_full.opt()])
            dense_phase(h_full, cfg.H, wa2_sb, xcat2, 2)
            asum2 = ald_build(xcat2, ald2, 2)
            edge_sweep(xcat2, asum2, 2)

    nc.compile()
    return nc


def _run(cfg, inputs, run_fn):
    prep = host_prep(cfg, inputs["edge_index"])
    wa1 = _weights_cat(np.asarray(inputs["W1"], np.float32),
                       np.asarray(inputs["a_src1"], np.float32),
                       np.asarray(inputs["a_dst1"], np.float32),
                       cfg.HEADS, cfg.H)
    wa2 = _weights_cat(np.asarray(inputs["W2"], np.float32),
                       np.asarray(inputs["a_src2"], np.float32),
                       np.asarray(inputs["a_dst2"], np.float32),
                       cfg.HEADS, cfg.OUT)
    mconst = np.tile(np.arange(128, dtype=np.float32)[None, :], (128, 1))
    ident = np.eye(128, dtype=np.float16)
    pio2 = np.stack([np.arange(128, dtype=np.float16),
                     np.ones(128, dtype=np.float16)])
    x16 = np.zeros((cfg.R, cfg.IN), dtype=np.float16)
    xf = np.asarray(inputs["x"], np.float32).astype(np.float16)
    x16[0:cfg.GAP_AT] = xf[0:cfg.GAP_AT]
    x16[cfg.GAP_AT + cfg.GAP:cfg.GAP_AT + cfg.GAP + (cfg.N - cfg.GAP_AT)] = \
        xf[cfg.GAP_AT:]
    x16t = np.ascontiguousarray(x16.T)

    nc = build_kernel(cfg, prep["C_lo"], prep["C_hi"], prep["C"],
                      prep["off"], prep["TOT"])
    in_maps = []
    for c in range(cfg.NCORES):
        in_maps.append({
            "x16t": x16t, "wa1": wa1, "wa2": wa2, "mconst": mconst,
            "ident": ident,
            "gidx": prep["gidx"][c], "dst2": prep["dst2"][c],
            "pio2": pio2, "dstloc": prep["dstloc"][c],
            "aldc_lo": prep["aldc_lo"][c], "aldc_hi": prep["aldc_hi"][c],
        })
    results = run_fn(nc, in_maps)
    out = np.concatenate([results[c]["out_slice"]
                          for c in range(cfg.NCORES)], axis=0)
    return out[:cfg.N]


def kernel(**inputs) -> np.ndarray:
    cfg = FULL

    def run_fn(nc, in_maps):
        res = run_bass_kernel_spmd(
            nc, in_maps, core_ids=list(range(cfg.NCORES)),
            trace=os.environ.get("GAT_TRACE", "0") == "1")
        if res.exec_time_ns is not None:
            print(f"HW exec time: {res.exec_time_ns} ns")
        if res.instructions_and_trace is not None:
            print(f"trace path: {res.instructions_and_trace[1]}")
        return res.results

    return _run(cfg, inputs, run_fn)


# revision 21
# speedup vs baseline: 1.0583x; 1.0583x over previous
"""2-layer GAT (heads=4, concat=False, ELU between) on 8 Trainium2 cores.

Design v2 (batched edge sweep, no one-hot transpose machinery):

Row space: 51200 rows = nodes 0..32639 | 1024 zero rows | nodes 32640..50175.
  row(n) = n + 1024*(n >= 32640). The zero gap gives every int16 gather
  window a known-zero row for junk indices.
xcat row (768B, 384 f16): [xh 256 | als 4 | ald 4 | pad 120].

Per layer:
- Dense phase (replicated): 50 batches x 1024 rows, xT via transpose-DMA,
  8 matmuls vs packed [fin,264] weights, PSUM -> xcat f16 rows.
- ald_loc build: two gathers (lo/hi windows of xcat cols 256:384, per-core
  indices select this core's 6272 slots; junk side hits zero rows), DVE add,
  store ald_loc [6272, 128] f16 (slot-major).
- Edge sweep per dst tile (128 nodes, C[t] chunks of 128 edges):
  3 gathers: G src rows (768B, lo/hi) + per-edge ald (256B rows of
  ald_loc tile window, idx = local dst).
  Batched attention: alpha = als+ald (TT), lrelu (STT), broadcast-exp on
  ACT -> wexp [128,C,4,64] f16, G *= wexp in place (TT), w -> cols 260:264.
  Per chunk: one f16 is_equal one-hot + one f16 matmul
  agg[128,264] += sel^T @ G[:,c,0:264] (denominators ride cols 260:264).
  Epilogue: recip*0.25, 4 ACT head scales, reduce over heads,
  ELU = max(s, exp(min(s,0))-1). h AllGathered as f16 [*,64].
"""
import sys
import os

sys.path.insert(0, '/opt/pypackages')
sys.path.insert(0, '/opt/trn_rl_repo')

import numpy as np

import concourse.bacc as bacc
import concourse.mybir as mybir
import concourse.tile as tile
from concourse.bass_utils import run_bass_kernel_spmd

F16 = mybir.dt.float16
F32 = mybir.dt.float32
I16 = mybir.dt.int16

NEG_SLOPE = 0.2


class Cfg:
    def __init__(self):
        self.N = 50000
        self.IN = 128
        self.H = 64
        self.OUT = 64
        self.HEADS = 4
        self.NCORES = 8
        self.T = 49                      # dst tiles per core
        self.NPC = self.T * 128          # 6272 slots per core
        self.NP = self.NCORES * self.NPC  # 50176 slots
        self.GAP_AT = 25472              # zero-gap insertion point (rows)
        self.GAP = 1024
        self.R = self.NP + self.GAP      # 51200 xcat rows
        self.SPLIT = 25600               # row-space int16 split
        self.ROW = 384                   # f16 elems per row (768B)


FULL = Cfg()


def _row(n):
    """node/slot id -> xcat row id (insert zero gap)."""
    n = np.asarray(n, dtype=np.int64)
    return n + np.where(n >= FULL.GAP_AT, FULL.GAP, 0)


def _wrap16(idx):
    """[n] int array (n%16==0) -> [128, n//16] int16 gather layout."""
    n = len(idx)
    base = np.asarray(idx, dtype=np.int16).reshape(n // 16, 16).T
    return np.tile(base, (8, 1))


def host_prep(cfg, edge_index):
    src = np.asarray(edge_index[0], dtype=np.int64)
    dst = np.asarray(edge_index[1], dtype=np.int64)
    loops = np.arange(cfg.N, dtype=np.int64)
    src = np.concatenate([src, loops])
    dst = np.concatenate([dst, loops])

    core_of = dst // cfg.NPC
    tile_of = (dst % cfg.NPC) // 128

    order = np.lexsort((src, tile_of, core_of))
    src_s, dst_s = src[order], dst[order]
    key = core_of[order] * cfg.T + tile_of[order]
    starts = np.searchsorted(key, np.arange(cfg.NCORES * cfg.T), side='left')
    ends = np.searchsorted(key, np.arange(cfg.NCORES * cfg.T), side='right')

    C_lo = np.zeros(cfg.T, dtype=np.int64)
    C_hi = np.zeros(cfg.T, dtype=np.int64)
    lists = {}
    for c in range(cfg.NCORES):
        for t in range(cfg.T):
            k = c * cfg.T + t
            es, ed = src_s[starts[k]:ends[k]], dst_s[starts[k]:ends[k]]
            lo = es < cfg.GAP_AT
            lists[(c, t)] = (es[lo], ed[lo], es[~lo], ed[~lo])
            C_lo[t] = max(C_lo[t], (int(np.sum(lo)) + 127) // 128)
            C_hi[t] = max(C_hi[t], (len(es) - int(np.sum(lo)) + 127) // 128)
    C_lo = np.maximum(C_lo, 1)
    C_hi = np.maximum(C_hi, 1)
    C = C_lo + C_hi
    TOT = int(C.sum())

    gidx = np.zeros((cfg.NCORES, 128, TOT * 8), dtype=np.int16)
    dst2 = np.zeros((cfg.NCORES, 2, TOT * 128), dtype=np.float16)
    dst2[:, 0, :] = 1.0
    dstloc = np.full((cfg.NCORES, 128, TOT), -1.0, dtype=np.float32)
    off = np.zeros(cfg.T + 1, dtype=np.int64)
    off[1:] = np.cumsum(C)
    for c in range(cfg.NCORES):
        for t in range(cfg.T):
            base = (c * cfg.T + t) * 128
            es_lo, ed_lo, es_hi, ed_hi = lists[(c, t)]
            nlo, nhi = int(C_lo[t]) * 128, int(C_hi[t]) * 128
            gi = np.zeros(nlo + nhi, dtype=np.int64)
            gi[:len(es_lo)] = es_lo                      # row(src)=src (<32640)
            gi[nlo:nlo + len(es_hi)] = es_hi + cfg.GAP - cfg.SPLIT
            dl = np.full(nlo + nhi, -1.0, dtype=np.float32)
            dl[:len(ed_lo)] = ed_lo - base
            dl[nlo:nlo + len(ed_hi)] = ed_hi - base
            o = int(off[t])
            ct = int(C[t])
            gidx[c, :, o * 8:(o + ct) * 8] = _wrap16(gi)
            dst2[c, 1, o * 128:(o + ct) * 128] = (-dl).astype(np.float16)
            dstloc[c, :, o:o + ct] = \
                dl.reshape(ct, 128).T.astype(np.float32)

    # per-core ald_loc build indices: slot i -> node c*NPC+i
    aldc_lo = np.zeros((cfg.NCORES, 128, cfg.NPC // 16), dtype=np.int16)
    aldc_hi = np.zeros((cfg.NCORES, 128, cfg.NPC // 16), dtype=np.int16)
    for c in range(cfg.NCORES):
        nodes = c * cfg.NPC + np.arange(cfg.NPC)
        is_lo = nodes < cfg.GAP_AT
        ilo = np.where(is_lo, nodes, cfg.GAP_AT)          # junk -> zero row
        ihi = np.where(is_lo, 0, nodes + cfg.GAP - cfg.SPLIT)
        aldc_lo[c] = _wrap16(ilo)
        aldc_hi[c] = _wrap16(ihi)
    return dict(C_lo=C_lo, C_hi=C_hi, C=C, off=off, TOT=TOT,
                gidx=gidx, dst2=dst2, dstloc=dstloc,
                aldc_lo=aldc_lo, aldc_hi=aldc_hi)


def _weights_cat(W, a_src, a_dst, heads, ch):
    """[Fin, heads*ch] + [heads, ch]x2 -> fp16 [Fin, 264]."""
    fin = W.shape[0]
    ws = np.einsum('fhc,hc->fh', W.reshape(fin, heads, ch), a_src)
    wd = np.einsum('fhc,hc->fh', W.reshape(fin, heads, ch), a_dst)
    out = np.zeros((fin, 264), dtype=np.float16)
    out[:, :heads * ch] = W.astype(np.float16)
    out[:, 256:260] = ws.astype(np.float16)
    out[:, 260:264] = wd.astype(np.float16)
    return out


def build_kernel(cfg, C_lo, C_hi, C, off, TOT):
    nc = bacc.Bacc("TRN2", target_bir_lowering=False, debug=False,
                   num_devices=cfg.NCORES, num_swdge_queues=4)
    R = cfg.R
    NB = R // 1024  # 50 dense batches

    x_in = nc.dram_tensor("x16t", [cfg.IN, R], F16, kind="ExternalInput")
    wa1 = nc.dram_tensor("wa1", [cfg.IN, 264], F16, kind="ExternalInput")
    wa2 = nc.dram_tensor("wa2", [cfg.H, 264], F16, kind="ExternalInput")
    mconst = nc.dram_tensor("mconst", [128, 128], F32, kind="ExternalInput")
    ident = nc.dram_tensor("ident", [128, 128], F16, kind="ExternalInput")
    gidx_d = nc.dram_tensor("gidx", [128, TOT * 8], I16, kind="ExternalInput")
    dst2_d = nc.dram_tensor("dst2", [2, TOT * 128], F16,
                            kind="ExternalInput")
    pio2 = nc.dram_tensor("pio2", [2, 128], F16, kind="ExternalInput")
    dstloc_d = nc.dram_tensor("dstloc", [128, TOT], F32,
                              kind="ExternalInput")
    aldc_lo_d = nc.dram_tensor("aldc_lo", [128, cfg.NPC // 16], I16,
                               kind="ExternalInput")
    aldc_hi_d = nc.dram_tensor("aldc_hi", [128, cfg.NPC // 16], I16,
                               kind="ExternalInput")
    out_d = nc.dram_tensor("out_slice", [cfg.NPC, cfg.OUT], F32,
                           kind="ExternalOutput")

    with tile.TileContext(nc) as tc:
        with tc.tile_pool(name="dram", bufs=1, space="DRAM") as dpool, \
             tc.tile_pool(name="const", bufs=1) as cpool, \
             tc.tile_pool(name="dense", bufs=3) as dnpool, \
             tc.tile_pool(name="aldb", bufs=1) as abpool, \
             tc.tile_pool(name="work", bufs=3) as pool, \
             tc.tile_pool(name="gpool", bufs=4) as gpool, \
             tc.tile_pool(name="apool", bufs=3) as apool, \
             tc.tile_pool(name="wpool", bufs=3) as wpool, \
             tc.tile_pool(name="seld", bufs=6) as sdpool, \
             tc.tile_pool(name="psA", bufs=2, space="PSUM") as psA, \
             tc.tile_pool(name="psB", bufs=2, space="PSUM") as psB, \
             tc.tile_pool(name="psU", bufs=1, space="PSUM") as psU, \
             tc.tile_pool(name="psL", bufs=1, space="PSUM") as psL:

            xcat1 = dpool.tile([R, cfg.ROW], F16, name="xcat1",
                               uniquify=False)
            xcat2 = dpool.tile([R, cfg.ROW], F16, name="xcat2",
                               uniquify=False)
            ald1 = dpool.tile([cfg.NPC, 128], F16, name="ald1",
                              uniquify=False)
            ald2 = dpool.tile([cfg.NPC, 128], F16, name="ald2",
                              uniquify=False)
            h_loc = dpool.tile([64, cfg.NPC], F16, name="h_loc",
                               uniquify=False)
            h_full = dpool.tile([cfg.NCORES, 64, cfg.NPC], F16,
                                name="h_full", uniquify=False,
                                addr_space="Shared")

            mconst_sb = cpool.tile([128, 128], F32)
            nc.sync.dma_start(out=mconst_sb[:], in_=mconst[:, :])
            ident_sb = cpool.tile([128, 128], F16)
            nc.sync.dma_start(out=ident_sb[:], in_=ident[:, :])
            wa1_sb = cpool.tile([cfg.IN, 264], F16)
            nc.sync.dma_start(out=wa1_sb[:], in_=wa1[:, :])
            wa2_sb = cpool.tile([cfg.H, 264], F16)
            nc.sync.dma_start(out=wa2_sb[:], in_=wa2[:, :])
            aldc_lo_sb = cpool.tile([128, cfg.NPC // 16], I16)
            nc.sync.dma_start(out=aldc_lo_sb[:], in_=aldc_lo_d[:, :])
            aldc_hi_sb = cpool.tile([128, cfg.NPC // 16], I16)
            nc.sync.dma_start(out=aldc_hi_sb[:], in_=aldc_hi_d[:, :])
            pio2_sb = cpool.tile([2, 128], F16)
            nc.sync.dma_start(out=pio2_sb[:], in_=pio2[:, :])
            zero_sb = cpool.tile([128, 1024], F16)
            nc.gpsimd.memset(zero_sb[:], 0)

            def dense_phase(src_h, fin, wa_sb, xcat, layer):
                """layer1: x_in is [IN, R] pre-transposed; layer2: h_full
                is [8, 64, NPC] (slot-major columns)."""
                GA, GE = cfg.GAP_AT, cfg.GAP_AT + cfg.GAP
                for b in range(NB):
                    nb = b * 1024
                    xT = dnpool.tile([128, 1024], F16, name=f"xT{layer}_{b}",
                                     tag="xT")
                    if layer == 1:
                        nc.sync.dma_start(out=xT[0:fin, :],
                                          in_=x_in[:, nb:nb + 1024])
                    else:
                        # xcat row j -> slot j (j<GA), zero (GA<=j<GE),
                        # else j-GAP; slots map to h_full[c, :, local]
                        segs = []
                        j = nb
                        while j < nb + 1024:
                            if j < GA:
                                n = min(GA, nb + 1024) - j
                                segs.append((j - nb, n, j))
                            elif j < GE:
                                n = min(GE, nb + 1024) - j
                                segs.append((j - nb, n, None))
                            else:
                                n = nb + 1024 - j
                                segs.append((j - nb, n, j - cfg.GAP))
                            j += n
                        for (xo, n, s0) in segs:
                            if s0 is None:
                                nc.vector.tensor_copy(
                                    xT[0:fin, xo:xo + n],
                                    zero_sb[0:fin, 0:n])
                                continue
                            while n > 0:
                                c0 = s0 // cfg.NPC
                                l0 = s0 - c0 * cfg.NPC
                                nn = min(n, cfg.NPC - l0)
                                nc.sync.dma_start(
                                    out=xT[0:fin, xo:xo + nn],
                                    in_=src_h[c0, :, l0:l0 + nn])
                                xo += nn
                                s0 += nn
                                n -= nn
                    xc = dnpool.tile([128, 8, 264], F16,
                                     name=f"xc{layer}_{b}", tag="xc")
                    for s in range(8):
                        ps = psA.tile([128, 264], F32,
                                      name=f"dp{layer}_{b}_{s}", tag="dps")
                        nc.tensor.matmul(
                            ps[:], xT[0:fin, s * 128:(s + 1) * 128],
                            wa_sb[:, :], start=True, stop=True)
                        nc.scalar.activation(
                            xc[:, s, :], ps[:, :],
                            mybir.ActivationFunctionType.Copy)
                    for half in range(2):
                        nc.sync.dma_start(
                            out=xcat[nb + half * 512:nb + half * 512 + 512,
                                     0:264].rearrange(
                                "(s p) d -> p s d", p=128),
                            in_=xc[:, half * 4:(half + 1) * 4, :])

            def ald_build(xcat, ald_loc, layer):
                """Collect this core's slots' [als|ald|pad] windows."""
                a_lo = abpool.tile([128, cfg.T, 128], F16,
                                   name=f"alo{layer}", tag="alo")
                nc.gpsimd.dma_gather(
                    a_lo[:], xcat[0:cfg.SPLIT, 256:384], aldc_lo_sb[:],
                    cfg.NPC, cfg.NPC, 128, elem_step=cfg.ROW,
                    single_packet=False, queue_num=0)
                a_hi = abpool.tile([128, cfg.T, 128], F16,
                                   name=f"ahi{layer}", tag="ahi")
                nc.gpsimd.dma_gather(
                    a_hi[:], xcat[cfg.SPLIT:R, 256:384], aldc_hi_sb[:],
                    cfg.NPC, cfg.NPC, 128, elem_step=cfg.ROW,
                    single_packet=False, queue_num=1)
                a_sum = abpool.tile([128, cfg.T, 128], F16,
                                    name=f"asm{layer}", tag="asm")
                nc.vector.tensor_tensor(out=a_sum[:], in0=a_lo[:],
                                        in1=a_hi[:],
                                        op=mybir.AluOpType.add)
                return a_sum

            def edge_sweep(xcat, a_sum, layer):
                for t in range(cfg.T):
                    q = (2 * t) % 4
                    q2 = (2 * t + 1) % 4
                    sfx = f"_{layer}_{t}"
                    Ct = int(C[t])
                    Clo = int(C_lo[t])
                    o = int(off[t])
                    idx_t = pool.tile([128, Ct * 8], I16, name="ix" + sfx,
                                      tag="ix")
                    nc.sync.dma_start(out=idx_t[:],
                                      in_=gidx_d[:, o * 8:(o + Ct) * 8])
                    d2_t = pool.tile([2, Ct * 128], F16, name="d2" + sfx,
                                     tag="d2")
                    nc.sync.dma_start(
                        out=d2_t[:],
                        in_=dst2_d[:, o * 128:(o + Ct) * 128])
                    dst_t = pool.tile([128, Ct], F32, name="dl" + sfx,
                                      tag="dl")
                    nc.sync.dma_start(out=dst_t[:],
                                      in_=dstloc_d[:, o:o + Ct])

                    G = gpool.tile([128, Ct, cfg.ROW], F16, name="G" + sfx,
                                   tag="G")
                    nc.gpsimd.dma_gather(
                        G[:, 0:Clo, :], xcat[0:cfg.SPLIT, :],
                        idx_t[:, 0:Clo * 8], Clo * 128, Clo * 128,
                        cfg.ROW, single_packet=False, queue_num=q)
                    nc.gpsimd.dma_gather(
                        G[:, Clo:Ct, :], xcat[cfg.SPLIT:R, :],
                        idx_t[:, Clo * 8:], (Ct - Clo) * 128,
                        (Ct - Clo) * 128,
                        cfg.ROW, single_packet=False, queue_num=q2)
                    # U[d,e] = d - dst[e] via K=2 matmul; SDT = (U == 0)
                    PC = ((Ct + 2) // 3) * 128
                    pieces = []
                    pb = 0
                    while pb < Ct * 128:
                        pieces.append((pb, min(PC, Ct * 128 - pb)))
                        pb += PC
                    sdt = apool.tile([128, Ct * 128], F16, name="st" + sfx,
                                     tag="st")
                    for pi, (pb, pn) in enumerate(pieces):
                        if pn <= 0:
                            continue
                        ups = psU.tile([128, 896], F32,
                                       name=f"up{sfx}_{pi}", tag="up")
                        for n0 in range(0, pn, 512):
                            nn = min(512, pn - n0)
                            nc.tensor.matmul(
                                ups[:, n0:n0 + nn], pio2_sb[:],
                                d2_t[:, pb + n0:pb + n0 + nn],
                                start=True, stop=True,
                                skip_group_check=True)
                        nc.vector.tensor_scalar(
                            sdt[:, pb:pb + pn], ups[:, 0:pn], 0.0, None,
                            mybir.AluOpType.is_equal)
                    ald_ps = psL.tile([128, Ct * 4], F32, name="ap" + sfx,
                                      tag="ap")
                    for c in range(Ct):
                        nc.tensor.matmul(
                            ald_ps[:, c * 4:(c + 1) * 4],
                            sdt[:, c * 128:(c + 1) * 128],
                            a_sum[:, t, 4:8], start=True, stop=True,
                            skip_group_check=True)
                    alde = pool.tile([128, Ct, 4], F16, name="ae" + sfx,
                                     tag="ae")
                    nc.vector.tensor_copy(
                        alde[:], ald_ps[:].rearrange("p (c f) -> p c f",
                                                     f=4))

                    alpha = pool.tile([128, Ct, 4], F32, name="al" + sfx,
                                      tag="al")
                    nc.vector.tensor_tensor(
                        out=alpha[:], in0=G[:, :, 256:260],
                        in1=alde[:], op=mybir.AluOpType.add)
                    wpre = pool.tile([128, Ct, 4], F32, name="wp" + sfx,
                                     tag="wp")
                    nc.vector.scalar_tensor_tensor(
                        out=wpre[:], in0=alpha[:], scalar=NEG_SLOPE,
                        in1=alpha[:], op0=mybir.AluOpType.mult,
                        op1=mybir.AluOpType.max)
                    wexp = wpool.tile([128, Ct, 4, 64], F16, name="we" + sfx,
                                      tag="we")
                    nc.scalar.activation(
                        wexp[:], wpre[:].unsqueeze(3).to_broadcast(
                            [128, Ct, 4, 64]),
                        mybir.ActivationFunctionType.Exp)
                    nc.vector.tensor_copy(
                        G[:, :, 260:264], wexp[:, :, :, 0])
                    nc.vector.tensor_tensor(
                        out=G[:, :, 0:256].rearrange(
                            "p c (h f) -> p c h f", h=4),
                        in0=G[:, :, 0:256].rearrange(
                            "p c (h f) -> p c h f", h=4),
                        in1=wexp[:], op=mybir.AluOpType.mult)

                    agg = psB.tile([128, 264], F32, name="agg" + sfx,
                                   tag="agg")
                    for c in range(Ct):
                        sel = sdpool.tile([128, 128], F16,
                                          name=f"sd{sfx}_{c}", tag="sd")
                        nc.vector.tensor_scalar(
                            sel[:], mconst_sb[:], dst_t[:, c:c + 1], None,
                            mybir.AluOpType.is_equal)
                        nc.tensor.matmul(
                            agg[:], sel[:], G[:, c, 0:264],
                            start=(c == 0), stop=(c == Ct - 1),
                            skip_group_check=True)

                    den = pool.tile([128, 4], F32, name="dn" + sfx, tag="dn")
                    nc.vector.tensor_scalar(den[:], agg[:, 260:264], 1e-16,
                                            None, mybir.AluOpType.max)
                    rec = pool.tile([128, 4], F32, name="rc" + sfx, tag="rc")
                    nc.vector.reciprocal(rec[:], den[:])
                    nc.vector.tensor_scalar(rec[:], rec[:], 0.25, None,
                                            mybir.AluOpType.mult)
                    tmp = pool.tile([128, 64, 4], F32, name="tm" + sfx,
                                    tag="tm")
                    for h in range(4):
                        nc.scalar.activation(
                            tmp[:, :, h], agg[:, h * 64:(h + 1) * 64],
                            mybir.ActivationFunctionType.Copy,
                            scale=rec[:, h:h + 1])
                    s0 = pool.tile([128, 64], F32, name="s0" + sfx, tag="s0")
                    nc.vector.tensor_reduce(
                        s0[:], tmp[:],
                        mybir.AxisListType.X, mybir.AluOpType.add)
                    if layer == 1:
                        ng = pool.tile([128, 64], F32, name="ng" + sfx,
                                       tag="ng")
                        nc.vector.tensor_scalar(ng[:], s0[:], 0.0, None,
                                                mybir.AluOpType.min)
                        ex = pool.tile([128, 64], F32, name="ex" + sfx,
                                       tag="ex")
                        nc.scalar.activation(
                            ex[:], ng[:], mybir.ActivationFunctionType.Exp)
                        hc = pool.tile([128, 64], F16, name="hc" + sfx,
                                       tag="hc")
                        nc.vector.scalar_tensor_tensor(
                            out=hc[:], in0=ex[:], scalar=1.0,
                            in1=s0[:],
                            op0=mybir.AluOpType.subtract,
                            op1=mybir.AluOpType.max)
                        trp = psL.tile([64, 128], F16, name="tr" + sfx,
                                       tag="tr")
                        nc.tensor.transpose(trp[:], hc[:], ident_sb[:])
                        hT = pool.tile([64, 128], F16, name="hT" + sfx,
                                       tag="hT")
                        nc.vector.tensor_copy(hT[:], trp[:])
                        nc.sync.dma_start(
                            out=h_loc[:, t * 128:(t + 1) * 128], in_=hT[:])
                    else:
                        nc.sync.dma_start(
                            out=out_d[t * 128:(t + 1) * 128, :], in_=s0[:])

            dense_phase(None, cfg.IN, wa1_sb, xcat1, 1)
            asum1 = ald_build(xcat1, ald1, 1)
            edge_sweep(xcat1, asum1, 1)
            nc.gpsimd.collective_compute(
                "AllGather", mybir.AluOpType.bypass,
                replica_groups=[list(range(cfg.NCORES))],
                ins=[h_loc.opt()], outs=[h_full.opt()])
            dense_phase(h_full, cfg.H, wa2_sb, xcat2, 2)
            asum2 = ald_build(xcat2, ald2, 2)
            edge_sweep(xcat2, asum2, 2)

    nc.compile()
    return nc


def _run(cfg, inputs, run_fn):
    prep = host_prep(cfg, inputs["edge_index"])
    wa1 = _weights_cat(np.asarray(inputs["W1"], np.float32),
                       np.asarray(inputs["a_src1"], np.float32),
                       np.asarray(inputs["a_dst1"], np.float32),
                       cfg.HEADS, cfg.H)
    wa2 = _weights_cat(np.asarray(inputs["W2"], np.float32),
                       np.asarray(inputs["a_src2"], np.float32),
                       np.asarray(inputs["a_dst2"], np.float32),
                       cfg.HEADS, cfg.OUT)
    mconst = np.tile(np.arange(128, dtype=np.float32)[None, :], (128, 1))
    ident = np.eye(128, dtype=np.float16)
    pio2 = np.stack([np.arange(128, dtype=np.float16),
                     np.ones(128, dtype=np.float16)])
    x16 = np.zeros((cfg.R, cfg.IN), dtype=np.float16)
    xf = np.asarray(inputs["x"], np.float32).astype(np.float16)
    x16[0:cfg.GAP_AT] = xf[0:cfg.GAP_AT]
    x16[cfg.GAP_AT + cfg.GAP:cfg.GAP_AT + cfg.GAP + (cfg.N - cfg.GAP_AT)] = \
        xf[cfg.GAP_AT:]
    x16t = np.ascontiguousarray(x16.T)

    nc = build_kernel(cfg, prep["C_lo"], prep["C_hi"], prep["C"],
                      prep["off"], prep["TOT"])
    in_maps = []
    for c in range(cfg.NCORES):
        in_maps.append({
            "x16t": x16t, "wa1": wa1, "wa2": wa2, "mconst": mconst,
            "ident": ident,
            "gidx": prep["gidx"][c], "dst2": prep["dst2"][c],
            "pio2": pio2, "dstloc": prep["dstloc"][c],
            "aldc_lo": prep["aldc_lo"][c], "aldc_hi": prep["aldc_hi"][c],
        })
    results = run_fn(nc, in_maps)
    out = np.concatenate([results[c]["out_slice"]
                          for c in range(cfg.NCORES)], axis=0)
    return out[:cfg.N]


def kernel(**inputs) -> np.ndarray:
    cfg = FULL

    def run_fn(nc, in_maps):
        res = run_bass_kernel_spmd(
            nc, in_maps, core_ids=list(range(cfg.NCORES)),
            trace=os.environ.get("GAT_TRACE", "0") == "1")
        if res.exec_time_ns is not None:
            print(f"HW exec time: {res.exec_time_ns} ns")
        if res.instructions_and_trace is not None:
            print(f"trace path: {res.instructions_and_trace[1]}")
        return res.results

    return _run(cfg, inputs, run_fn)


# revision 22
# speedup vs baseline: 1.0768x; 1.0175x over previous
"""2-layer GAT (heads=4, concat=False, ELU between) on 8 Trainium2 cores.

Design v2 (batched edge sweep, no one-hot transpose machinery):

Row space: 51200 rows = nodes 0..32639 | 1024 zero rows | nodes 32640..50175.
  row(n) = n + 1024*(n >= 32640). The zero gap gives every int16 gather
  window a known-zero row for junk indices.
xcat row (768B, 384 f16): [xh 256 | als 4 | ald 4 | pad 120].

Per layer:
- Dense phase (replicated): 50 batches x 1024 rows, xT via transpose-DMA,
  8 matmuls vs packed [fin,264] weights, PSUM -> xcat f16 rows.
- ald_loc build: two gathers (lo/hi windows of xcat cols 256:384, per-core
  indices select this core's 6272 slots; junk side hits zero rows), DVE add,
  store ald_loc [6272, 128] f16 (slot-major).
- Edge sweep per dst tile (128 nodes, C[t] chunks of 128 edges):
  3 gathers: G src rows (768B, lo/hi) + per-edge ald (256B rows of
  ald_loc tile window, idx = local dst).
  Batched attention: alpha = als+ald (TT), lrelu (STT), broadcast-exp on
  ACT -> wexp [128,C,4,64] f16, G *= wexp in place (TT), w -> cols 260:264.
  Per chunk: one f16 is_equal one-hot + one f16 matmul
  agg[128,264] += sel^T @ G[:,c,0:264] (denominators ride cols 260:264).
  Epilogue: recip*0.25, 4 ACT head scales, reduce over heads,
  ELU = max(s, exp(min(s,0))-1). h AllGathered as f16 [*,64].
"""
import sys
import os

sys.path.insert(0, '/opt/pypackages')
sys.path.insert(0, '/opt/trn_rl_repo')

import numpy as np

import concourse.bacc as bacc
import concourse.mybir as mybir
import concourse.tile as tile
from concourse.bass_utils import run_bass_kernel_spmd

F16 = mybir.dt.float16
F32 = mybir.dt.float32
I16 = mybir.dt.int16

NEG_SLOPE = 0.2


class Cfg:
    def __init__(self):
        self.N = 50000
        self.IN = 128
        self.H = 64
        self.OUT = 64
        self.HEADS = 4
        self.NCORES = 8
        self.T = 49                      # dst tiles per core
        self.NPC = self.T * 128          # 6272 slots per core
        self.NP = self.NCORES * self.NPC  # 50176 slots
        self.GAP_AT = 25472              # zero-gap insertion point (rows)
        self.GAP = 1024
        self.R = self.NP + self.GAP      # 51200 xcat rows
        self.SPLIT = 25600               # row-space int16 split
        self.ROW = 384                   # f16 elems per row (768B)


FULL = Cfg()


def _row(n):
    """node/slot id -> xcat row id (insert zero gap)."""
    n = np.asarray(n, dtype=np.int64)
    return n + np.where(n >= FULL.GAP_AT, FULL.GAP, 0)


def _wrap16(idx):
    """[n] int array (n%16==0) -> [128, n//16] int16 gather layout."""
    n = len(idx)
    base = np.asarray(idx, dtype=np.int16).reshape(n // 16, 16).T
    return np.tile(base, (8, 1))


def host_prep(cfg, edge_index):
    src = np.asarray(edge_index[0], dtype=np.int64)
    dst = np.asarray(edge_index[1], dtype=np.int64)
    loops = np.arange(cfg.N, dtype=np.int64)
    src = np.concatenate([src, loops])
    dst = np.concatenate([dst, loops])

    core_of = dst // cfg.NPC
    tile_of = (dst % cfg.NPC) // 128

    order = np.lexsort((src, tile_of, core_of))
    src_s, dst_s = src[order], dst[order]
    key = core_of[order] * cfg.T + tile_of[order]
    starts = np.searchsorted(key, np.arange(cfg.NCORES * cfg.T), side='left')
    ends = np.searchsorted(key, np.arange(cfg.NCORES * cfg.T), side='right')

    C_lo = np.zeros(cfg.T, dtype=np.int64)
    C_hi = np.zeros(cfg.T, dtype=np.int64)
    lists = {}
    for c in range(cfg.NCORES):
        for t in range(cfg.T):
            k = c * cfg.T + t
            es, ed = src_s[starts[k]:ends[k]], dst_s[starts[k]:ends[k]]
            lo = es < cfg.GAP_AT
            lists[(c, t)] = (es[lo], ed[lo], es[~lo], ed[~lo])
            C_lo[t] = max(C_lo[t], (int(np.sum(lo)) + 127) // 128)
            C_hi[t] = max(C_hi[t], (len(es) - int(np.sum(lo)) + 127) // 128)
    C_lo = np.maximum(C_lo, 1)
    C_hi = np.maximum(C_hi, 1)
    C = C_lo + C_hi
    TOT = int(C.sum())

    gidx = np.zeros((cfg.NCORES, 128, TOT * 8), dtype=np.int16)
    dst2 = np.zeros((cfg.NCORES, 2, TOT * 128), dtype=np.float16)
    dst2[:, 0, :] = 1.0
    dstloc = np.full((cfg.NCORES, 128, TOT), -1.0, dtype=np.float32)
    off = np.zeros(cfg.T + 1, dtype=np.int64)
    off[1:] = np.cumsum(C)
    for c in range(cfg.NCORES):
        for t in range(cfg.T):
            base = (c * cfg.T + t) * 128
            es_lo, ed_lo, es_hi, ed_hi = lists[(c, t)]
            nlo, nhi = int(C_lo[t]) * 128, int(C_hi[t]) * 128
            gi = np.zeros(nlo + nhi, dtype=np.int64)
            gi[:len(es_lo)] = es_lo                      # row(src)=src (<32640)
            gi[nlo:nlo + len(es_hi)] = es_hi + cfg.GAP - cfg.SPLIT
            dl = np.full(nlo + nhi, -1.0, dtype=np.float32)
            dl[:len(ed_lo)] = ed_lo - base
            dl[nlo:nlo + len(ed_hi)] = ed_hi - base
            o = int(off[t])
            ct = int(C[t])
            gidx[c, :, o * 8:(o + ct) * 8] = _wrap16(gi)
            dst2[c, 1, o * 128:(o + ct) * 128] = (-dl).astype(np.float16)
            dstloc[c, :, o:o + ct] = \
                dl.reshape(ct, 128).T.astype(np.float32)

    # per-core ald_loc build indices: slot i -> node c*NPC+i
    aldc_lo = np.zeros((cfg.NCORES, 128, cfg.NPC // 16), dtype=np.int16)
    aldc_hi = np.zeros((cfg.NCORES, 128, cfg.NPC // 16), dtype=np.int16)
    for c in range(cfg.NCORES):
        nodes = c * cfg.NPC + np.arange(cfg.NPC)
        is_lo = nodes < cfg.GAP_AT
        ilo = np.where(is_lo, nodes, cfg.GAP_AT)          # junk -> zero row
        ihi = np.where(is_lo, 0, nodes + cfg.GAP - cfg.SPLIT)
        aldc_lo[c] = _wrap16(ilo)
        aldc_hi[c] = _wrap16(ihi)
    return dict(C_lo=C_lo, C_hi=C_hi, C=C, off=off, TOT=TOT,
                gidx=gidx, dst2=dst2, dstloc=dstloc,
                aldc_lo=aldc_lo, aldc_hi=aldc_hi)


def _weights_cat(W, a_src, a_dst, heads, ch):
    """[Fin, heads*ch] + [heads, ch]x2 -> fp16 [Fin, 264]."""
    fin = W.shape[0]
    ws = np.einsum('fhc,hc->fh', W.reshape(fin, heads, ch), a_src)
    wd = np.einsum('fhc,hc->fh', W.reshape(fin, heads, ch), a_dst)
    out = np.zeros((fin, 264), dtype=np.float16)
    out[:, :heads * ch] = W.astype(np.float16)
    out[:, 256:260] = ws.astype(np.float16)
    out[:, 260:264] = wd.astype(np.float16)
    return out


def build_kernel(cfg, C_lo, C_hi, C, off, TOT):
    nc = bacc.Bacc("TRN2", target_bir_lowering=False, debug=False,
                   num_devices=cfg.NCORES, num_swdge_queues=4)
    R = cfg.R
    NB = R // 1024  # 50 dense batches

    x_in = nc.dram_tensor("x16t", [cfg.IN, R], F16, kind="ExternalInput")
    wa1 = nc.dram_tensor("wa1", [cfg.IN, 264], F16, kind="ExternalInput")
    wa2 = nc.dram_tensor("wa2", [cfg.H, 264], F16, kind="ExternalInput")
    mconst = nc.dram_tensor("mconst", [128, 128], F32, kind="ExternalInput")
    ident = nc.dram_tensor("ident", [128, 128], F16, kind="ExternalInput")
    gidx_d = nc.dram_tensor("gidx", [128, TOT * 8], I16, kind="ExternalInput")
    dst2_d = nc.dram_tensor("dst2", [2, TOT * 128], F16,
                            kind="ExternalInput")
    pio2 = nc.dram_tensor("pio2", [2, 128], F16, kind="ExternalInput")
    dstloc_d = nc.dram_tensor("dstloc", [128, TOT], F32,
                              kind="ExternalInput")
    aldc_lo_d = nc.dram_tensor("aldc_lo", [128, cfg.NPC // 16], I16,
                               kind="ExternalInput")
    aldc_hi_d = nc.dram_tensor("aldc_hi", [128, cfg.NPC // 16], I16,
                               kind="ExternalInput")
    out_d = nc.dram_tensor("out_slice", [cfg.NPC, cfg.OUT], F32,
                           kind="ExternalOutput")

    with tile.TileContext(nc) as tc:
        with tc.tile_pool(name="dram", bufs=1, space="DRAM") as dpool, \
             tc.tile_pool(name="const", bufs=1) as cpool, \
             tc.tile_pool(name="dense", bufs=3) as dnpool, \
             tc.tile_pool(name="aldb", bufs=1) as abpool, \
             tc.tile_pool(name="work", bufs=3) as pool, \
             tc.tile_pool(name="gpool", bufs=5) as gpool, \
             tc.tile_pool(name="apool", bufs=3) as apool, \
             tc.tile_pool(name="wpool", bufs=3) as wpool, \
             tc.tile_pool(name="seld", bufs=6) as sdpool, \
             tc.tile_pool(name="psA", bufs=2, space="PSUM") as psA, \
             tc.tile_pool(name="psB", bufs=2, space="PSUM") as psB, \
             tc.tile_pool(name="psU", bufs=1, space="PSUM") as psU, \
             tc.tile_pool(name="psL", bufs=1, space="PSUM") as psL:

            xcat1 = dpool.tile([R, cfg.ROW], F16, name="xcat1",
                               uniquify=False)
            xcat2 = dpool.tile([R, cfg.ROW], F16, name="xcat2",
                               uniquify=False)
            ald1 = dpool.tile([cfg.NPC, 128], F16, name="ald1",
                              uniquify=False)
            ald2 = dpool.tile([cfg.NPC, 128], F16, name="ald2",
                              uniquify=False)
            h_loc = dpool.tile([64, cfg.NPC], F16, name="h_loc",
                               uniquify=False)
            h_full = dpool.tile([cfg.NCORES, 64, cfg.NPC], F16,
                                name="h_full", uniquify=False,
                                addr_space="Shared")

            mconst_sb = cpool.tile([128, 128], F32)
            nc.sync.dma_start(out=mconst_sb[:], in_=mconst[:, :])
            ident_sb = cpool.tile([128, 128], F16)
            nc.sync.dma_start(out=ident_sb[:], in_=ident[:, :])
            wa1_sb = cpool.tile([cfg.IN, 264], F16)
            nc.sync.dma_start(out=wa1_sb[:], in_=wa1[:, :])
            wa2_sb = cpool.tile([cfg.H, 264], F16)
            nc.sync.dma_start(out=wa2_sb[:], in_=wa2[:, :])
            aldc_lo_sb = cpool.tile([128, cfg.NPC // 16], I16)
            nc.sync.dma_start(out=aldc_lo_sb[:], in_=aldc_lo_d[:, :])
            aldc_hi_sb = cpool.tile([128, cfg.NPC // 16], I16)
            nc.sync.dma_start(out=aldc_hi_sb[:], in_=aldc_hi_d[:, :])
            pio2_sb = cpool.tile([2, 128], F16)
            nc.sync.dma_start(out=pio2_sb[:], in_=pio2[:, :])
            zero_sb = cpool.tile([128, 1024], F16)
            nc.gpsimd.memset(zero_sb[:], 0)

            def dense_phase(src_h, fin, wa_sb, xcat, layer):
                """layer1: x_in is [IN, R] pre-transposed; layer2: h_full
                is [8, 64, NPC] (slot-major columns)."""
                GA, GE = cfg.GAP_AT, cfg.GAP_AT + cfg.GAP
                for b in range(NB):
                    nb = b * 1024
                    xT = dnpool.tile([128, 1024], F16, name=f"xT{layer}_{b}",
                                     tag="xT")
                    if layer == 1:
                        nc.sync.dma_start(out=xT[0:fin, :],
                                          in_=x_in[:, nb:nb + 1024])
                    else:
                        # xcat row j -> slot j (j<GA), zero (GA<=j<GE),
                        # else j-GAP; slots map to h_full[c, :, local]
                        segs = []
                        j = nb
                        while j < nb + 1024:
                            if j < GA:
                                n = min(GA, nb + 1024) - j
                                segs.append((j - nb, n, j))
                            elif j < GE:
                                n = min(GE, nb + 1024) - j
                                segs.append((j - nb, n, None))
                            else:
                                n = nb + 1024 - j
                                segs.append((j - nb, n, j - cfg.GAP))
                            j += n
                        for (xo, n, s0) in segs:
                            if s0 is None:
                                nc.vector.tensor_copy(
                                    xT[0:fin, xo:xo + n],
                                    zero_sb[0:fin, 0:n])
                                continue
                            while n > 0:
                                c0 = s0 // cfg.NPC
                                l0 = s0 - c0 * cfg.NPC
                                nn = min(n, cfg.NPC - l0)
                                nc.sync.dma_start(
                                    out=xT[0:fin, xo:xo + nn],
                                    in_=src_h[c0, :, l0:l0 + nn])
                                xo += nn
                                s0 += nn
                                n -= nn
                    xc = dnpool.tile([128, 8, 264], F16,
                                     name=f"xc{layer}_{b}", tag="xc")
                    for s in range(8):
                        ps = psA.tile([128, 264], F32,
                                      name=f"dp{layer}_{b}_{s}", tag="dps")
                        nc.tensor.matmul(
                            ps[:], xT[0:fin, s * 128:(s + 1) * 128],
                            wa_sb[:, :], start=True, stop=True)
                        if s % 2 == 0:
                            nc.scalar.activation(
                                xc[:, s, :], ps[:, :],
                                mybir.ActivationFunctionType.Copy)
                        else:
                            nc.vector.tensor_copy(xc[:, s, :], ps[:, :])
                    for half in range(2):
                        nc.sync.dma_start(
                            out=xcat[nb + half * 512:nb + half * 512 + 512,
                                     0:264].rearrange(
                                "(s p) d -> p s d", p=128),
                            in_=xc[:, half * 4:(half + 1) * 4, :])

            def ald_build(xcat, ald_loc, layer):
                """Collect this core's slots' [als|ald|pad] windows."""
                a_lo = abpool.tile([128, cfg.T, 128], F16,
                                   name=f"alo{layer}", tag="alo")
                nc.gpsimd.dma_gather(
                    a_lo[:], xcat[0:cfg.SPLIT, 256:384], aldc_lo_sb[:],
                    cfg.NPC, cfg.NPC, 128, elem_step=cfg.ROW,
                    single_packet=False, queue_num=0)
                a_hi = abpool.tile([128, cfg.T, 128], F16,
                                   name=f"ahi{layer}", tag="ahi")
                nc.gpsimd.dma_gather(
                    a_hi[:], xcat[cfg.SPLIT:R, 256:384], aldc_hi_sb[:],
                    cfg.NPC, cfg.NPC, 128, elem_step=cfg.ROW,
                    single_packet=False, queue_num=1)
                a_sum = abpool.tile([128, cfg.T, 128], F16,
                                    name=f"asm{layer}", tag="asm")
                nc.vector.tensor_tensor(out=a_sum[:], in0=a_lo[:],
                                        in1=a_hi[:],
                                        op=mybir.AluOpType.add)
                return a_sum

            def edge_sweep(xcat, a_sum, layer):
                for t in range(cfg.T):
                    q = (2 * t) % 4
                    q2 = (2 * t + 1) % 4
                    sfx = f"_{layer}_{t}"
                    Ct = int(C[t])
                    Clo = int(C_lo[t])
                    o = int(off[t])
                    idx_t = pool.tile([128, Ct * 8], I16, name="ix" + sfx,
                                      tag="ix")
                    nc.sync.dma_start(out=idx_t[:],
                                      in_=gidx_d[:, o * 8:(o + Ct) * 8])
                    d2_t = pool.tile([2, Ct * 128], F16, name="d2" + sfx,
                                     tag="d2")
                    nc.sync.dma_start(
                        out=d2_t[:],
                        in_=dst2_d[:, o * 128:(o + Ct) * 128])
                    dst_t = pool.tile([128, Ct], F32, name="dl" + sfx,
                                      tag="dl")
                    nc.sync.dma_start(out=dst_t[:],
                                      in_=dstloc_d[:, o:o + Ct])

                    G = gpool.tile([128, Ct, cfg.ROW], F16, name="G" + sfx,
                                   tag="G")
                    nc.gpsimd.dma_gather(
                        G[:, 0:Clo, :], xcat[0:cfg.SPLIT, :],
                        idx_t[:, 0:Clo * 8], Clo * 128, Clo * 128,
                        cfg.ROW, single_packet=False, queue_num=q)
                    nc.gpsimd.dma_gather(
                        G[:, Clo:Ct, :], xcat[cfg.SPLIT:R, :],
                        idx_t[:, Clo * 8:], (Ct - Clo) * 128,
                        (Ct - Clo) * 128,
                        cfg.ROW, single_packet=False, queue_num=q2)
                    # U[d,e] = d - dst[e] via K=2 matmul; SDT = (U == 0)
                    PC = ((Ct + 2) // 3) * 128
                    pieces = []
                    pb = 0
                    while pb < Ct * 128:
                        pieces.append((pb, min(PC, Ct * 128 - pb)))
                        pb += PC
                    sdt = apool.tile([128, Ct * 128], F16, name="st" + sfx,
                                     tag="st")
                    for pi, (pb, pn) in enumerate(pieces):
                        if pn <= 0:
                            continue
                        ups = psU.tile([128, 896], F32,
                                       name=f"up{sfx}_{pi}", tag="up")
                        for n0 in range(0, pn, 512):
                            nn = min(512, pn - n0)
                            nc.tensor.matmul(
                                ups[:, n0:n0 + nn], pio2_sb[:],
                                d2_t[:, pb + n0:pb + n0 + nn],
                                start=True, stop=True,
                                skip_group_check=True)
                        nc.vector.tensor_scalar(
                            sdt[:, pb:pb + pn], ups[:, 0:pn], 0.0, None,
                            mybir.AluOpType.is_equal)
                    ald_ps = psL.tile([128, Ct * 4], F32, name="ap" + sfx,
                                      tag="ap")
                    for c in range(Ct):
                        nc.tensor.matmul(
                            ald_ps[:, c * 4:(c + 1) * 4],
                            sdt[:, c * 128:(c + 1) * 128],
                            a_sum[:, t, 4:8], start=True, stop=True,
                            skip_group_check=True)
                    alde = pool.tile([128, Ct, 4], F16, name="ae" + sfx,
                                     tag="ae")
                    nc.vector.tensor_copy(
                        alde[:], ald_ps[:].rearrange("p (c f) -> p c f",
                                                     f=4))

                    alpha = pool.tile([128, Ct, 4], F32, name="al" + sfx,
                                      tag="al")
                    nc.vector.tensor_tensor(
                        out=alpha[:], in0=G[:, :, 256:260],
                        in1=alde[:], op=mybir.AluOpType.add)
                    wpre = pool.tile([128, Ct, 4], F32, name="wp" + sfx,
                                     tag="wp")
                    nc.vector.scalar_tensor_tensor(
                        out=wpre[:], in0=alpha[:], scalar=NEG_SLOPE,
                        in1=alpha[:], op0=mybir.AluOpType.mult,
                        op1=mybir.AluOpType.max)
                    wexp = wpool.tile([128, Ct, 4, 64], F16, name="we" + sfx,
                                      tag="we")
                    nc.scalar.activation(
                        wexp[:], wpre[:].unsqueeze(3).to_broadcast(
                            [128, Ct, 4, 64]),
                        mybir.ActivationFunctionType.Exp)
                    nc.vector.tensor_copy(
                        G[:, :, 260:264], wexp[:, :, :, 0])
                    nc.vector.tensor_tensor(
                        out=G[:, :, 0:256].rearrange(
                            "p c (h f) -> p c h f", h=4),
                        in0=G[:, :, 0:256].rearrange(
                            "p c (h f) -> p c h f", h=4),
                        in1=wexp[:], op=mybir.AluOpType.mult)

                    agg = psB.tile([128, 264], F32, name="agg" + sfx,
                                   tag="agg")
                    for c in range(Ct):
                        sel = sdpool.tile([128, 128], F16,
                                          name=f"sd{sfx}_{c}", tag="sd")
                        nc.vector.tensor_scalar(
                            sel[:], mconst_sb[:], dst_t[:, c:c + 1], None,
                            mybir.AluOpType.is_equal)
                        nc.tensor.matmul(
                            agg[:], sel[:], G[:, c, 0:264],
                            start=(c == 0), stop=(c == Ct - 1),
                            skip_group_check=True)

                    den = pool.tile([128, 4], F32, name="dn" + sfx, tag="dn")
                    nc.vector.tensor_scalar(den[:], agg[:, 260:264], 1e-16,
                                            None, mybir.AluOpType.max)
                    rec = pool.tile([128, 4], F32, name="rc" + sfx, tag="rc")
                    nc.vector.reciprocal(rec[:], den[:])
                    nc.vector.tensor_scalar(rec[:], rec[:], 0.25, None,
                                            mybir.AluOpType.mult)
                    tmp = pool.tile([128, 64, 4], F32, name="tm" + sfx,
                                    tag="tm")
                    for h in range(4):
                        nc.scalar.activation(
                            tmp[:, :, h], agg[:, h * 64:(h + 1) * 64],
                            mybir.ActivationFunctionType.Copy,
                            scale=rec[:, h:h + 1])
                    s0 = pool.tile([128, 64], F32, name="s0" + sfx, tag="s0")
                    nc.vector.tensor_reduce(
                        s0[:], tmp[:],
                        mybir.AxisListType.X, mybir.AluOpType.add)
                    if layer == 1:
                        ng = pool.tile([128, 64], F32, name="ng" + sfx,
                                       tag="ng")
                        nc.vector.tensor_scalar(ng[:], s0[:], 0.0, None,
                                                mybir.AluOpType.min)
                        ex = pool.tile([128, 64], F32, name="ex" + sfx,
                                       tag="ex")
                        nc.scalar.activation(
                            ex[:], ng[:], mybir.ActivationFunctionType.Exp)
                        hc = pool.tile([128, 64], F16, name="hc" + sfx,
                                       tag="hc")
                        nc.vector.scalar_tensor_tensor(
                            out=hc[:], in0=ex[:], scalar=1.0,
                            in1=s0[:],
                            op0=mybir.AluOpType.subtract,
                            op1=mybir.AluOpType.max)
                        trp = psL.tile([64, 128], F16, name="tr" + sfx,
                                       tag="tr")
                        nc.tensor.transpose(trp[:], hc[:], ident_sb[:])
                        hT = pool.tile([64, 128], F16, name="hT" + sfx,
                                       tag="hT")
                        nc.vector.tensor_copy(hT[:], trp[:])
                        nc.sync.dma_start(
                            out=h_loc[:, t * 128:(t + 1) * 128], in_=hT[:])
                    else:
                        nc.sync.dma_start(
                            out=out_d[t * 128:(t + 1) * 128, :], in_=s0[:])

            dense_phase(None, cfg.IN, wa1_sb, xcat1, 1)
            asum1 = ald_build(xcat1, ald1, 1)
            edge_sweep(xcat1, asum1, 1)
            nc.gpsimd.collective_compute(
                "AllGather", mybir.AluOpType.bypass,
                replica_groups=[list(range(cfg.NCORES))],
                ins=[h_loc.opt()], outs=[h_full.opt()])
            dense_phase(h_full, cfg.H, wa2_sb, xcat2, 2)
            asum2 = ald_build(xcat2, ald2, 2)
            edge_sweep(xcat2, asum2, 2)

    nc.compile()
    return nc


def _run(cfg, inputs, run_fn):
    prep = host_prep(cfg, inputs["edge_index"])
    wa1 = _weights_cat(np.asarray(inputs["W1"], np.float32),
                       np.asarray(inputs["a_src1"], np.float32),
                       np.asarray(inputs["a_dst1"], np.float32),
                       cfg.HEADS, cfg.H)
    wa2 = _weights_cat(np.asarray(inputs["W2"], np.float32),
                       np.asarray(inputs["a_src2"], np.float32),
                       np.asarray(inputs["a_dst2"], np.float32),
                       cfg.HEADS, cfg.OUT)
    mconst = np.tile(np.arange(128, dtype=np.float32)[None, :], (128, 1))
    ident = np.eye(128, dtype=np.float16)
    pio2 = np.stack([np.arange(128, dtype=np.float16),
                     np.ones(128, dtype=np.float16)])
    x16 = np.zeros((cfg.R, cfg.IN), dtype=np.float16)
    xf = np.asarray(inputs["x"], np.float32).astype(np.float16)
    x16[0:cfg.GAP_AT] = xf[0:cfg.GAP_AT]
    x16[cfg.GAP_AT + cfg.GAP:cfg.GAP_AT + cfg.GAP + (cfg.N - cfg.GAP_AT)] = \
        xf[cfg.GAP_AT:]
    x16t = np.ascontiguousarray(x16.T)

    nc = build_kernel(cfg, prep["C_lo"], prep["C_hi"], prep["C"],
                      prep["off"], prep["TOT"])
    in_maps = []
    for c in range(cfg.NCORES):
        in_maps.append({
            "x16t": x16t, "wa1": wa1, "wa2": wa2, "mconst": mconst,
            "ident": ident,
            "gidx": prep["gidx"][c], "dst2": prep["dst2"][c],
            "pio2": pio2, "dstloc": prep["dstloc"][c],
            "aldc_lo": prep["aldc_lo"][c], "aldc_hi": prep["aldc_hi"][c],
        })
    results = run_fn(nc, in_maps)
    out = np.concatenate([results[c]["out_slice"]
                          for c in range(cfg.NCORES)], axis=0)
    return out[:cfg.N]


def kernel(**inputs) -> np.ndarray:
    cfg = FULL

    def run_fn(nc, in_maps):
        res = run_bass_kernel_spmd(
            nc, in_maps, core_ids=list(range(cfg.NCORES)),
            trace=os.environ.get("GAT_TRACE", "0") == "1")
        if res.exec_time_ns is not None:
            print(f"HW exec time: {res.exec_time_ns} ns")
        if res.instructions_and_trace is not None:
            print(f"trace path: {res.instructions_and_trace[1]}")
        return res.results

    return _run(cfg, inputs, run_fn)
